# revision 27
# baseline (speedup 1.0000x reference)
"""Trainium2 Bass kernel for CDRExtractor (segment_reduce).

Input : segmentation_mask (64, 3, 512, 512) fp32
Output: (64, 5) fp32 = [cdr, disc_mean, cup_mean, disc_mean, cup_mean]

Sharding: pure data parallel, 8 samples per core across 8 cores; each core
streams its 24 MiB shard once (DMA roofline ~70us/core at ~358 GB/s).

Per-core algorithm (e-space formulation; 16 tiles of 2 samples x 128 rows):
  E = exp(x) for all 3 channels       (ACT, one (128,3072) pass, bf16 out)
  MM = [max(e1,e0)|max(e2,e0)]        (DVE TT max, broadcast-e0 AP, 2x bf16)
  s = e0+e1+e2                        (GPSIMD TT adds - only add/mult legal
                                       on Pool in this toolchain)
  r = exp(-ln(s))                     (ACT Ln+Exp; ACT Reciprocal/Rsqrt are
                                       banned for accuracy, and both funcs
                                       live in one act table set ->
                                       single table load)
  row-sum(e_c*r), row-count[e_c>MM]   (DVE scalar_tensor_tensor, fused
                                       elementwise+reduce into fp32 accum
                                       columns; count>0.5 == row contains
                                       argmax==label, exact)
  tail: PE transpose + ones-matmul over the (128,32) accumulators,
        iota+penalty reduce-min/max for ymin/ymax per (sample,label),
        heights = relu(ymax-ymin), cdr = h_cup/(h_disc+1e-6), means /= H*W.

Engine busy per core (CoreSim cost model): DVE ~94us (pacing), ACT ~77us,
DMA ~76us, Pool ~65us; end-to-end sim ~113us.
"""

import numpy as np
from contextlib import ExitStack

B, C, H, W = 64, 3, 512, 512
NCORES = 8
SPC = B // NCORES      # samples per core = 8
PAIRS = SPC // 2       # sample pairs per core = 4
NB = H // 128          # 128-row blocks = 4
HW = float(H * W)

_CACHE = {}


def _build():
    import concourse.bass as bass
    import concourse.bacc as bacc
    import concourse.mybir as mybir
    from concourse.tile import TileContext

    # Offer only the act-table set containing BOTH exp and ln (ids kept
    # aligned with act_info.json) so the table never reloads mid-kernel.
    if not _CACHE.get("act_patch"):
        _orig_tables = bacc.get_activation_tables

        def _only_ln_exp(arch):
            t = _orig_tables(arch)
            keep = "natural_log_exp_and_others"
            return {k: (v if k == keep else set()) for k, v in t.items()}

        bacc.get_activation_tables = _only_ln_exp
        _CACHE["act_patch"] = True

    f32 = mybir.dt.float32
    bf16 = mybir.dt.bfloat16
    Alu = mybir.AluOpType
    AFT = mybir.ActivationFunctionType
    X_AX = mybir.AxisListType.X

    nc = bacc.Bacc()
    x = nc.dram_tensor("x", (SPC, C, H, W), f32, kind="ExternalInput")
    iota_in = nc.dram_tensor("iota", (32, 128), f32, kind="ExternalInput")
    ident_in = nc.dram_tensor("ident", (128, 128), f32, kind="ExternalInput")
    ones_in = nc.dram_tensor("ones", (128, 1), f32, kind="ExternalInput")
    out = nc.dram_tensor("out", (5, SPC), f32, kind="ExternalOutput")

    with TileContext(nc) as tc, ExitStack() as ctx:
        cpool = ctx.enter_context(tc.tile_pool(name="consts", bufs=1))
        apool = ctx.enter_context(tc.tile_pool(name="accs", bufs=1))
        mpool = ctx.enter_context(tc.tile_pool(name="main", bufs=4))
        ppool = ctx.enter_context(tc.tile_pool(name="ps", bufs=1, space="PSUM"))

        iota = cpool.tile([32, 128], f32, tag="iota")
        nc.sync.dma_start(iota[:, :], iota_in[:, :])
        ident = cpool.tile([128, 128], f32, tag="ident")
        nc.sync.dma_start(ident[:, :], ident_in[:, :])
        ones = cpool.tile([128, 1], f32, tag="ones")
        nc.sync.dma_start(ones[:, :], ones_in[:, :])

        # accumulators: col j = b*8 + s
        RS1 = apool.tile([128, 32], f32, tag="RS1")  # row-sums of p1 (cup)
        RS2 = apool.tile([128, 32], f32, tag="RS2")  # row-sums of p2 (disc)
        DM1 = apool.tile([128, 32], f32, tag="DM1")  # row-max argmax margin lbl1
        DM2 = apool.tile([128, 32], f32, tag="DM2")

        def stage_a(t, b):
            X = mpool.tile([128, 2 * C * W], f32, tag="X", name=f"X_{t}_{b}",
                           bufs=5)
            E = mpool.tile([128, 2 * C * W], bf16, tag="E", name=f"E_{t}_{b}")
            if (t, b) == (0, 0):
                # fill-latency: small per-(sample,channel) DMAs + half exps
                for si in range(2):
                    for ci in range(C):
                        src = x[2 * t + si, ci, b * 128:(b + 1) * 128, :]
                        off = (si * C + ci) * W
                        nc.sync.dma_start(X[:, off:off + W], src)
                    half = slice(si * C * W, (si + 1) * C * W)
                    nc.scalar.activation(E[:, half], X[:, half], AFT.Exp)
                return E
            src = x[2 * t:2 * t + 2, :, b * 128:(b + 1) * 128, :]
            src = src.rearrange("s c h w -> h s c w")
            Xv = X.rearrange("p (s c w) -> p s c w", s=2, c=C)
            nc.sync.dma_start(Xv, src)
            nc.scalar.activation(E[:, :], X[:, :], AFT.Exp)
            return E

        def stage_b1(t, b, E):
            """DVE/POOL work after exp: maxes + denominator."""
            Ev = E.rearrange("p (s x) -> p s x", s=2)
            e0v = Ev[:, :, 0:512]
            e1v = Ev[:, :, 512:1024]
            e2v = Ev[:, :, 1024:1536]

            # channel maxes vs e0 (argmax margins), one DVE instr:
            # MM = [max(e1,e0) | max(e2,e0)] per sample
            El = E.rearrange("p (s l w) -> p s l w", s=2, l=C)
            MM = mpool.tile([128, 2048], bf16, tag="MM", name=f"MM_{t}_{b}")
            MMv = MM.rearrange("p (s l w) -> p s l w", s=2, l=2)
            nc.vector.tensor_tensor(
                MMv, El[:, :, 1:3, :],
                El[:, :, 0:1, :].broadcast_to((128, 2, 2, 512)), Alu.max)

            s01 = mpool.tile([128, 1024], bf16, tag="s01", name=f"s01_{t}_{b}")
            s01v = s01.rearrange("p (s w) -> p s w", s=2)
            nc.gpsimd.tensor_tensor(s01v, e0v, e1v, Alu.add)
            sden = mpool.tile([128, 1024], bf16, tag="sden",
                              name=f"sden_{t}_{b}")
            sdenv = sden.rearrange("p (s w) -> p s w", s=2)
            nc.gpsimd.tensor_tensor(sdenv, s01v, e2v, Alu.add)
            return MM, sden

        def stage_b2(t, b, E, MM, sden):
            """ACT r = exp(-ln(s)), then the fused sum/count STTs."""
            lns = mpool.tile([128, 1024], bf16, tag="lns", name=f"lns_{t}_{b}")
            nc.scalar.activation(lns[:, :], sden[:, :], AFT.Ln)
            rb = mpool.tile([128, 1024], bf16, tag="rb", name=f"rb_{t}_{b}")
            nc.scalar.activation(rb[:, :], lns[:, :], AFT.Exp, scale=-1.0)

            scr = mpool.tile([128, 2048], bf16, tag="scr", name=f"scr_{t}_{b}")
            for si in range(2):
                s_g = 2 * t + si
                col = b * 8 + s_g
                base = si * C * W
                e1 = E[:, base + 512:base + 1024]
                e2 = E[:, base + 1024:base + 1536]
                rbs = rb[:, si * 512:(si + 1) * 512]
                m01s = MM[:, si * 1024:si * 1024 + 512]
                m02s = MM[:, si * 1024 + 512:si * 1024 + 1024]
                # p-sums: out = (e+0)*r, accum = row-sum
                nc.vector.scalar_tensor_tensor(
                    scr[:, 0:512], e1, 0.0, rbs,
                    Alu.add, Alu.mult, accum_out=RS1[:, col:col + 1])
                nc.vector.scalar_tensor_tensor(
                    scr[:, 512:1024], e2, 0.0, rbs,
                    Alu.add, Alu.mult, accum_out=RS2[:, col:col + 1])
                # argmax presence: out = [e > max(other two)], accum =
                # per-row count of argmax==label pixels
                nc.vector.scalar_tensor_tensor(
                    scr[:, 1024:1536], e1, 0.0, m02s,
                    Alu.add, Alu.is_gt, accum_out=DM1[:, col:col + 1])
                nc.vector.scalar_tensor_tensor(
                    scr[:, 1536:2048], e2, 0.0, m01s,
                    Alu.add, Alu.is_gt, accum_out=DM2[:, col:col + 1])

        # software pipeline: emit tile i's ln/rexp/STT stage after tile
        # i+1's exp so ACT never stalls waiting for the DVE denominator.
        tiles = [(t, b) for t in range(PAIRS) for b in range(NB)]
        pending = None  # (t, b, E, MM, sden)
        for (t, b) in tiles:
            E = stage_a(t, b)
            MM, sden = stage_b1(t, b, E)
            if pending is not None:
                stage_b2(*pending)
            pending = (t, b, E, MM, sden)
        stage_b2(*pending)

        # ---- tail ----
        O = cpool.tile([1, 40], f32, tag="O")
        S12 = ppool.tile([1, 64], f32, tag="S12")
        nc.tensor.matmul(S12[:, 0:32], ones[:, :], RS1[:, :], start=True, stop=True)
        nc.tensor.matmul(S12[:, 32:64], ones[:, :], RS2[:, :], start=True, stop=True)

        heights = []
        for li, DM in enumerate((DM1, DM2)):
            TD = ppool.tile([32, 128], f32, tag=f"TD{li}")
            nc.tensor.transpose(TD[:, :], DM[:, :], ident[:, :])
            TL = cpool.tile([32, 128], f32, tag=f"TL{li}")
            nc.vector.tensor_copy(TL[:, :], TD[:, :])
            pen = cpool.tile([32, 128], f32, tag=f"pen{li}")
            nc.vector.tensor_scalar(pen[:, :], TL[:, :], 0.5, 1e6,
                                    Alu.is_lt, Alu.mult)
            cmin = cpool.tile([32, 128], f32, tag=f"cmin{li}")
            nc.vector.tensor_tensor(cmin[:, :], pen[:, :], iota[:, :], Alu.add)
            cmax = cpool.tile([32, 128], f32, tag=f"cmax{li}")
            nc.vector.tensor_tensor(cmax[:, :], iota[:, :], pen[:, :],
                                    Alu.subtract)
            Y = cpool.tile([32, 2], f32, tag=f"Y{li}")
            nc.vector.tensor_reduce(Y[:, 0:1], cmin[:, :], X_AX, op=Alu.min)
            nc.vector.tensor_reduce(Y[:, 1:2], cmax[:, :], X_AX, op=Alu.max)
            YTmin = ppool.tile([1, 32], f32, tag=f"YTmin{li}")
            YTmax = ppool.tile([1, 32], f32, tag=f"YTmax{li}")
            nc.tensor.transpose(YTmin[:, :], Y[:, 0:1], ident[0:32, 0:32])
            nc.tensor.transpose(YTmax[:, :], Y[:, 1:2], ident[0:32, 0:32])
            ymin8 = cpool.tile([1, 8], f32, tag=f"ymin{li}")
            ymax8 = cpool.tile([1, 8], f32, tag=f"ymax{li}")
            nc.vector.tensor_reduce(
                ymin8[:, :], YTmin[0:1, :].rearrange("p (b s) -> p s b", b=4),
                X_AX, op=Alu.min)
            nc.vector.tensor_reduce(
                ymax8[:, :], YTmax[0:1, :].rearrange("p (b s) -> p s b", b=4),
                X_AX, op=Alu.max)
            hL = cpool.tile([1, 8], f32, tag=f"h{li}")
            nc.vector.tensor_tensor(hL[:, :], ymax8[:, :], ymin8[:, :],
                                    Alu.subtract)
            nc.vector.tensor_scalar_max(hL[:, :], hL[:, :], 0.0)
            heights.append(hL)

        h_cup, h_disc = heights
        den = cpool.tile([1, 8], f32, tag="den")
        nc.vector.tensor_scalar_add(den[:, :], h_disc[:, :], 1e-6)
        rec = cpool.tile([1, 8], f32, tag="rec")
        nc.vector.reciprocal(rec[:, :], den[:, :])
        nc.vector.tensor_tensor(O[:, 0:8], h_cup[:, :], rec[:, :], Alu.mult)

        ms1 = cpool.tile([1, 8], f32, tag="ms1")
        ms2 = cpool.tile([1, 8], f32, tag="ms2")
        nc.vector.tensor_reduce(
            ms1[:, :], S12[0:1, 0:32].rearrange("p (b s) -> p s b", b=4),
            X_AX, op=Alu.add)
        nc.vector.tensor_reduce(
            ms2[:, :], S12[0:1, 32:64].rearrange("p (b s) -> p s b", b=4),
            X_AX, op=Alu.add)
        sc = 1.0 / HW
        nc.vector.tensor_scalar_mul(O[:, 8:16], ms2[:, :], sc)
        nc.vector.tensor_scalar_mul(O[:, 16:24], ms1[:, :], sc)
        nc.vector.tensor_scalar_mul(O[:, 24:32], ms2[:, :], sc)
        nc.vector.tensor_scalar_mul(O[:, 32:40], ms1[:, :], sc)

        nc.sync.dma_start(out[:, :], O[:, :])

    nc.finalize()
    return nc


def _get_nc():
    if "nc" not in _CACHE:
        _CACHE["nc"] = _build()
    return _CACHE["nc"]


def _host_inputs():
    iota = (np.arange(128, dtype=np.float32)[None, :]
            + 128.0 * np.repeat(np.arange(4, dtype=np.float32), 8)[:, None])
    ident = np.eye(128, dtype=np.float32)
    ones = np.ones((128, 1), dtype=np.float32)
    return iota, ident, ones


def _run(seg_mask, trace=False):
    from concourse.bass_utils import run_bass_kernel_spmd

    x = np.ascontiguousarray(np.asarray(seg_mask, dtype=np.float32))
    assert x.shape == (B, C, H, W)
    iota, ident, ones = _host_inputs()
    in_maps = [
        {"x": x[SPC * c:SPC * (c + 1)], "iota": iota, "ident": ident,
         "ones": ones}
        for c in range(NCORES)
    ]
    nc = _get_nc()
    res = run_bass_kernel_spmd(nc, in_maps, core_ids=list(range(NCORES)),
                               trace=trace)
    outs = []
    for c in range(NCORES):
        o = np.asarray(res.results[c]["out"]).reshape(5, SPC).T
        outs.append(o)
    full = np.concatenate(outs, axis=0).astype(np.float32)
    return full, res


def kernel(segmentation_mask):
    full, _ = _run(segmentation_mask, trace=False)
    return full
